# revision 49
# baseline (speedup 1.0000x reference)
"""Multi-head self-attention (1x1-conv QKV -> softmax attention -> 1x1-conv)
on Trainium2, 8 NeuronCores, data-parallel over (batch, query-half).

Problem (hardcoded): x[4,256,48,48], Wqkv[768,256], bqkv[768], W0[256,256],
b0[256]; heads=8, dim_head=32, n=2304 pixels.

Sharding: core = b*2 + half. Each core computes K/V for its whole image
(2304 keys) and attention + output projection for its 1152 queries.
No cross-core communication.

Per-core dataflow (bf16 matmul operands, fp32 PSUM accumulation —
HW-measured: bf16 is the only dtype that streams the PE at full rate;
fp32 is 2-pass and float32r single-pass runs at half clock, both ~2x
slower; fp8 DoubleRow PV measured no faster and rel-err ~1.7e-2):
  - x [256, 2304] bf16 (query half permuted first), weights bf16.
  - k_all [(m,d)=256, j] / q_all [(m,d)=256, i]: K=128-accum matmuls,
    bias via ACT Identity on the PSUM->SBUF copy (Wq,bq pre-scaled by
    d^-0.5 on host).
  - vT [j, 8*(32+2)]: per head 32 v-dims + ones col (den trick) + pad;
    V bias added via DVE tensor_add with a host-broadcast [128,NV] tile.
  - scores^T S_T[j, i] = k_m^T q_m per head: K=32 matmuls, head PAIRS
    row-packed via tile_position (rb, rb+32) — the pair runs
    CONCURRENTLY in different PE row groups; each matmul's output owns
    a full PSUM bank.  Query chunks (512, 512, 128) — narrow chunk last
    so the end-of-kernel pipeline drain is short.
  - P = exp(S_T) on ScalarE (ACT), bf16 out (no max subtraction: |s|<~6).
    (A custom-DVE polynomial exp (KDVE env) exists but measured slower:
    its 2.3us latency stalls the PE pipeline.)
  - out^T+den = [vT | 1]^T @ P: M=34 matmuls col-packed via tile_position
    (0,0)/(0,64) into one PSUM bank — the head pair again overlaps in
    different PE column groups.  Accumulate over the 18 key tiles; psum
    row 32 (head0) / 96 (head1) = softmax denominators.
  - normalize: den rows copied to partition 0, reciprocal_approx_fast,
    stream_shuffle broadcast across the 32-block, DVE multiply into outc
    (all DVE ops base-aligned).  od gather (SBUF->SBUF partition-remap
    DMA) emitted per pair right after its last chunk so it overlaps the
    remaining attention.
  - y = W0 @ od + b0: K=128-accum bf16 matmuls; b0 applied as ACT
    per-partition bias on the PSUM->SBUF copy.
Engine balance at ~234us: PE ~206us busy (out-column-rate bound:
QK+PV stream 332k psum columns ~= 138us floor + weight loads), ACT
~197us (softmax exp, 1 elem/cycle/partition floor = 138us), DVE ~53us.
Both engines sit at their structural floors; y written per chunk on the
sync DMA queue (gpsimd casting DMAs add a ~3.4us DGE drain at teardown
— avoided), prologue weight loads on the scalar DMA queue.
"""

import os as _os

import numpy as np

import concourse.bass as bass
import concourse.mybir as mybir
import concourse.tile as tile
from concourse import bacc
from concourse import bass_utils

F32 = mybir.dt.float32
BF16 = mybir.dt.bfloat16
R32 = mybir.dt.float32r
AF = mybir.ActivationFunctionType

B, C, HH, WW = 4, 256, 48, 48
HEADS, D = 8, 32
N = HH * WW            # 2304 keys per image
NCORES = 8
NQ = N // 2            # 1152 queries per core
JT = N // 128          # 18 key tiles
ICW = 512              # query chunk width; narrow chunk last (tail drain)
CHUNKS = [(0, 512), (512, 512), (1024, 128)]
NV = HEADS * (D + 2)   # 272: vT columns (32 v dims + ones col + zero pad
                       # per head; f32r matmul lhs free dim must be even)

DEBUG_STAGE = int(_os.environ.get("KSTAGE", "4"))
RECIP_MODE = _os.environ.get("KREC", "fast")
_DTMAP = {"f32": F32, "bf16": BF16, "f32r": R32, "f8e4": mybir.dt.float8e4}
QK_DT = _DTMAP[_os.environ.get("KQK", "bf16")]
PV_DT = _DTMAP[_os.environ.get("KPV", "bf16")]
IN_DT = _DTMAP[_os.environ.get("KIN", "bf16")]   # x / proj weights (matmul ins)
OUT_DT = _DTMAP[_os.environ.get("KOUT", "bf16")]  # outc / od / w0 (final matmul)
FP8PV = PV_DT == mybir.dt.float8e4


def _mm(ap):
    return ap


# ---- custom DVE exp: offload part of the softmax exp to the Vector engine.
# exp(s + b) == (alpha * p(s/32))^32 with p a minimax cubic for e^t on
# |t|<=0.25 and alpha = e^(b/32).  Two DVE ops: cubic+alpha (8 ALU stages),
# then five squarings (5 stages).  Max rel err ~1e-3 over |s|<=8.
DVE_EXP = int(_os.environ.get("KDVE", "0"))   # 0=off, N>0: every Nth j on DVE
_EXP_C2, _EXP_C3 = 0.5020893755368885, 0.16666274457238633
_DVE_OPS = {}


def _register_dve_exp():
    if _DVE_OPS:
        return _DVE_OPS
    from concourse.dve_spec import (
        Spec, Src0, C0, C1, C2, C3, One, sq, lower, _spill_c3_to_src1,
        _has_src1,
    )
    from concourse import dve_ops as dops
    from concourse.dve_uop import DveOpSpec

    def mk(name, spec):
        if name in dops._SUB_OPCODE_FOR_NAME:
            for op in dops.OPS:
                if op.name == name:
                    return op
        opcode = dops._CUSTOM_DVE_ROW_BASE + len(dops.OPS)
        shas = {}
        for ver in ("v3", "v4"):
            shas[ver] = DveOpSpec(
                name=name, opcode=opcode, uops=lower(spec, ver=ver),
                rd1_en=_has_src1(spec),
            ).sha(ver)
        op = dops.DveOp(name, spec, subdim=False, uops_sha=shas)
        dops.OPS.append(op)
        dops._SUB_OPCODE_FOR_NAME[name] = opcode
        return op

    t = Src0 * C0
    p = ((t * C1 + C2) * t + One) * t + One

    def ref1(in0, in1, s0, s1, imm2):
        tt = in0.astype(np.float32) * s0
        return (((tt * s1 + imm2) * tt + 1.0) * tt + 1.0) * in1

    op1 = mk("MHSA_EXPP", Spec(body=_spill_c3_to_src1(p * C3), reference=ref1))
    op2 = mk("MHSA_EXPS", Spec(
        body=sq(sq(sq(sq(sq(Src0))))),
        reference=lambda in0, in1, s0, s1, imm2: in0.astype(np.float32) ** 32,
    ))
    _DVE_OPS.update(op1=op1, op2=op2)
    return _DVE_OPS


def _chunks(total, step):
    out = []
    o = 0
    while o < total:
        w = min(step, total - o)
        out.append((o, w))
        o += w
    return out


def _body(tc, x_d, wq_d, bq_d, wk_d, bk_d, wv_d, wvb_d, w0_d, w0b_d, y_d):
    from contextlib import ExitStack

    nc = tc.nc
    with ExitStack() as ctx:
        const = ctx.enter_context(tc.tile_pool(name="const", bufs=1))
        data = ctx.enter_context(tc.tile_pool(name="data", bufs=1))

        # ---------------- load inputs ----------------
        x_sb = [const.tile([128, N], IN_DT, name=f"xa{t}", tag=f"xa{t}") for t in range(2)]

        def load2(name, dram, cols, dt=IN_DT):
            # row-half 0 on the sync queue, row-half 1 on the scalar queue:
            # descriptor generation serializes per queue, so mirroring the
            # halves across two queues halves every load's latency.
            ts_ = [const.tile([128, cols], dt, name=f"{name}{t}", tag=f"{name}{t}") for t in range(2)]
            nc.sync.dma_start(ts_[0][:], dram[0:128, :])
            nc.scalar.dma_start(ts_[1][:], dram[128:256, :])
            return ts_

        # issue order matters: the first K-proj matmul needs wk, x chunk 0
        # and bk — put those first, and split the two x row-halves across
        # the sync and vector DMA queues so descriptor generation runs in
        # parallel (it serializes per issuing queue).
        wk_sb = load2("wk", wk_d, C)
        for (o, w) in _chunks(N, 768):
            nc.sync.dma_start(x_sb[0][:, o:o + w], x_d[0:128, o:o + w])
            nc.gpsimd.dma_start(x_sb[1][:, o:o + w], x_d[128:256, o:o + w])
        bk_sb = load2("bk", bk_d, 1, dt=F32)
        wq_sb = load2("wq", wq_d, C)
        bq_sb = load2("bq", bq_d, 1, dt=F32)
        wv_sb = load2("wv", wv_d, NV)
        wvb_sb = const.tile([128, NV], F32, name="wvbias", tag="wvbias")
        nc.sync.dma_start(wvb_sb[:], wvb_d[:, :])
        w0_sb = load2("w0", w0_d, C, dt=OUT_DT)
        w0p_sb = load2("w0p", w0b_d, 1, dt=F32)

        # reciprocal staging tiles: rows 1-31 stay 1.0 forever (only row 0 is
        # rewritten by the per-chunk reciprocal), so memset once here.
        rt0 = const.tile([32, ICW], F32, name="rt0", tag="rt0")
        rt1 = const.tile([32, ICW], F32, name="rt1", tag="rt1")
        nc.vector.memset(rt0[:], 1.0)
        nc.vector.memset(rt1[:], 1.0)
        expb_sb = const.tile([128, 1], F32, name="expb", tag="expb")
        nc.vector.memset(expb_sb[:], -2.0)
        if DVE_EXP:
            _register_dve_exp()
            alpha_sb = const.tile([128, 1], F32, name="alpha", tag="alpha")
            nc.vector.memset(alpha_sb[:], float(np.exp(-2.0 / 32.0)) if FP8PV else 1.0)

        # persistent activations
        k_sb = [data.tile([128, N], QK_DT, name=f"k{g}", tag=f"k{g}") for g in range(2)]
        q_sb = [data.tile([128, NQ], QK_DT, name=f"q{g}", tag=f"q{g}") for g in range(2)]
        # fp8 DoubleRow path: vt for a j-PAIR lives in one [128, 2*NV] tile
        # (middle dim = j parity) so one matmul contracts 256 keys.
        # exp shift (see expb_sb): keeps exp(s) inside fp8e4 range
        if FP8PV:
            vt_sb = [data.tile([128, 2 * NV], PV_DT, name=f"vt{jp}", tag=f"vt{jp}")
                     for jp in range(JT // 2)]
        else:
            vt_sb = [data.tile([128, NV], PV_DT, name=f"vt{j}", tag=f"vt{j}") for j in range(JT)]
        # output tiles: fp8 path keeps one [32, NQ] tile per head (PV psum at
        # partition base 0); bf16 path keeps the packed pv layout (head pair
        # at partitions 0-31 / 64-95 of a [128, NQ] tile).
        if FP8PV:
            outc_sb = [data.tile([32, NQ], OUT_DT, name=f"oc{t}", tag=f"oc{t}")
                       for t in range(HEADS)]
        else:
            outc_sb = [data.tile([128, NQ], OUT_DT, name=f"oc{t}", tag=f"oc{t}")
                       for t in range(4)]
        y_sb = [data.tile([128, NQ], F32, name=f"y{g}", tag=f"y{g}") for g in range(2)]

        # ---------------- projections ----------------
        with tc.tile_pool(name="prj", bufs=2, space="PSUM") as prj:
            for hg in range(2):
                hsl = slice(hg * 128, (hg + 1) * 128)
                for (o, w) in _chunks(N, 512):
                    kps = prj.tile([128, 512], F32, name="kps", tag="kps")
                    nc.tensor.matmul(kps[:, :w], wk_sb[0][:, hsl], x_sb[0][:, o:o + w], start=True, stop=False)
                    nc.tensor.matmul(kps[:, :w], wk_sb[1][:, hsl], x_sb[1][:, o:o + w], start=False, stop=True)
                    nc.scalar.activation(k_sb[hg][:, o:o + w], kps[:, :w], AF.Identity, bias=bk_sb[hg][:, 0:1])
                for (o, w) in _chunks(NQ, 512):
                    qps = prj.tile([128, 512], F32, name="qps", tag="qps")
                    nc.tensor.matmul(qps[:, :w], wq_sb[0][:, hsl], x_sb[0][:, o:o + w], start=True, stop=False)
                    nc.tensor.matmul(qps[:, :w], wq_sb[1][:, hsl], x_sb[1][:, o:o + w], start=False, stop=True)
                    nc.scalar.activation(q_sb[hg][:, o:o + w], qps[:, :w], AF.Identity, bias=bq_sb[hg][:, 0:1])
            for j in range(JT):
                jsl = slice(j * 128, (j + 1) * 128)
                vps = prj.tile([128, NV], F32, name="vps", tag="vps")
                nc.tensor.matmul(vps[:], x_sb[0][:, jsl], wv_sb[0][:], start=True, stop=False)
                nc.tensor.matmul(vps[:], x_sb[1][:, jsl], wv_sb[1][:], start=False, stop=True)
                if FP8PV:
                    jo = j % 2
                    nc.vector.tensor_add(
                        vt_sb[j // 2][:, jo * NV:(jo + 1) * NV], vps[:], wvb_sb[:])
                else:
                    nc.vector.tensor_add(vt_sb[j][:], vps[:], wvb_sb[:])

        if DEBUG_STAGE < 2:
            for g in range(2):
                nc.vector.tensor_copy(y_sb[g][:], q_sb[g][:])
                nc.sync.dma_start(y_d[g * 128:(g + 1) * 128, :], y_sb[g][:])
            return

        od_sb = [data.tile([128, NQ], OUT_DT, name=f"od{g}", tag=f"od{g}") for g in range(2)]

        # ---------------- attention main loop ----------------
        # PSUM budget: st (3 or 2) bufs x 2 banks + pv banks = 8.
        STB = 2 if FP8PV else 3
        with tc.tile_pool(name="stp", bufs=STB, space="PSUM") as stp, \
             tc.tile_pool(name="pvp", bufs=2, space="PSUM") as pvp, \
             tc.tile_pool(name="pv1p", bufs=2, space="PSUM") as pv1p, \
             tc.tile_pool(name="ptp", bufs=4) as ptp, \
             tc.tile_pool(name="etp", bufs=2) as etp, \
             tc.tile_pool(name="epi", bufs=2) as epi:
            for hg in range(2):
                for pr in range(2):
                    rb = pr * 64       # partition base of this head pair
                    for (ic0, w) in CHUNKS:
                        # head pair packed into one PSUM bank via column
                        # tile_position: head0 at partitions 0:34, head1 at
                        # 64:98 — the two matmuls run concurrently in
                        # different PE column groups.
                        if FP8PV:
                            pv0 = pvp.tile([34, ICW], F32, name="pv0", tag="pv0")
                            pv1 = pv1p.tile([34, ICW], F32, name="pv1", tag="pv1")
                            pvs = (pv0, pv1)
                        else:
                            pv = pvp.tile([128, ICW], F32, name="pv", tag="pv")
                        pts = {}

                        if FP8PV:
                            # one DoubleRow matmul per j-pair: lhsT/rhs get a
                            # middle dim of 2 (j parity), contracting 256 keys.
                            def emit_pv(jp, w=w, pvs=pvs, pts=pts, hg=hg, pr=pr):
                                pt = pts.pop(jp)
                                p3 = pt[:].rearrange("p (o h q) -> p o h q", o=2, h=2)
                                v3 = vt_sb[jp][:].rearrange("p (o v) -> p o v", o=2)
                                for hl in range(2):
                                    gh = hg * 4 + 2 * pr + hl
                                    nc.tensor.matmul(
                                        pvs[hl][0:34, 0:w],
                                        v3[:, :, gh * 34:gh * 34 + 34],
                                        p3[:, :, hl, 0:w],
                                        start=(jp == 0), stop=(jp == JT // 2 - 1),
                                        perf_mode=mybir.MatmulPerfMode.DoubleRow,
                                    )
                        else:
                            def emit_pv(j, w=w, pv=pv, pts=pts, hg=hg, pr=pr):
                                pt = pts.pop(j)
                                for hl, base in enumerate((0, 64)):
                                    gh = hg * 4 + 2 * pr + hl
                                    nc.tensor.matmul(
                                        pv[base:base + 34, 0:w],
                                        _mm(vt_sb[j][:, gh * 34:gh * 34 + 34]),
                                        _mm(pt[:, hl * ICW:hl * ICW + w]),
                                        start=(j == 0), stop=(j == JT - 1),
                                        tile_position=(0, base),
                                    )

                        for j in range(JT):
                            st = stp.tile([128, 1024], F32, name="st", tag="st")
                            for hl in range(2):
                                nc.tensor.matmul(
                                    st[:, hl * 512:hl * 512 + w],
                                    _mm(k_sb[hg][rb + hl * 32:rb + (hl + 1) * 32, j * 128:(j + 1) * 128]),
                                    _mm(q_sb[hg][rb + hl * 32:rb + (hl + 1) * 32, ic0:ic0 + w]),
                                    start=True, stop=True,
                                    tile_position=(rb + hl * 32, 0),
                                )
                            use_dve = DVE_EXP > 0 and (j % DVE_EXP == DVE_EXP - 1)
                            src_v = st[:].rearrange("p (s q) -> p s q", s=2)[:, :, 0:w]
                            if FP8PV:
                                jo = j % 2
                                if jo == 0:
                                    pt = ptp.tile([128, 4 * ICW], PV_DT, name="pt", tag="pt")
                                    pts[j // 2] = pt
                                else:
                                    pt = pts[j // 2]
                                dst_v = (pt[:, jo * 2 * ICW:(jo + 1) * 2 * ICW]
                                         .rearrange("p (s q) -> p s q", s=2)[:, :, 0:w])
                            else:
                                pt = ptp.tile([128, 2 * ICW], PV_DT, name="pt", tag="pt")
                                dst_v = pt[:].rearrange("p (s q) -> p s q", s=2)[:, :, 0:w]
                            if use_dve:
                                et = etp.tile([128, 1024], F32, name="et", tag="et")
                                et_v = et[:].rearrange("p (s q) -> p s q", s=2)[:, :, 0:w]
                                nc.vector._custom_dve(
                                    _DVE_OPS["op1"], out=et_v, in0=src_v,
                                    in1=alpha_sb[:, 0:1], s0=1.0 / 32.0,
                                    s1=_EXP_C3, imm2=_EXP_C2)
                                nc.vector._custom_dve(
                                    _DVE_OPS["op2"], out=dst_v, in0=et_v)
                            elif FP8PV:
                                nc.scalar.activation(dst_v, src_v, AF.Exp, bias=expb_sb[:])
                            else:
                                nc.scalar.activation(dst_v, src_v, AF.Exp)
                            if FP8PV:
                                if jo == 1 and j // 2 >= 1:
                                    emit_pv(j // 2 - 1)
                            else:
                                pts[j] = pt
                                if j >= 1:
                                    emit_pv(j - 1)
                        emit_pv(JT - 1 if not FP8PV else JT // 2 - 1)

                        # epilogue: reciprocal of the den row at partition 0,
                        # stream_shuffle broadcast across the 32-block, DVE
                        # multiply into outc (all ops base-aligned).
                        dt0 = epi.tile([1, ICW], F32, name="dt0", tag="dt0")
                        dt1 = epi.tile([1, ICW], F32, name="dt1", tag="dt1")
                        if FP8PV:
                            oc0 = outc_sb[hg * 4 + 2 * pr]
                            oc1 = outc_sb[hg * 4 + 2 * pr + 1]
                            if DEBUG_STAGE < 3:
                                nc.vector.tensor_copy(oc0[0:32, ic0:ic0 + w], pv0[0:32, 0:w])
                                nc.vector.tensor_copy(oc1[0:32, ic0:ic0 + w], pv1[0:32, 0:w])
                                continue
                            nc.vector.tensor_copy(dt0[0:1, 0:w], pv0[32:33, 0:w])
                            nc.vector.tensor_copy(dt1[0:1, 0:w], pv1[32:33, 0:w])
                        else:
                            oc = outc_sb[hg * 2 + pr]
                            if DEBUG_STAGE < 3:
                                nc.vector.tensor_copy(oc[0:33, ic0:ic0 + w], pv[0:33, 0:w])
                                nc.vector.tensor_copy(oc[64:97, ic0:ic0 + w], pv[64:97, 0:w])
                                continue
                            nc.vector.tensor_copy(dt0[0:1, 0:w], pv[32:33, 0:w])
                            nc.vector.tensor_copy(dt1[0:1, 0:w], pv[96:97, 0:w])
                        if RECIP_MODE == "fast":
                            nc.vector.reciprocal_approx_fast(rt0[0:1, 0:w], dt0[0:1, 0:w])
                            nc.vector.reciprocal_approx_fast(rt1[0:1, 0:w], dt1[0:1, 0:w])
                        else:
                            nc.vector.reciprocal(rt0[0:1, 0:w], dt0[0:1, 0:w])
                            nc.vector.reciprocal(rt1[0:1, 0:w], dt1[0:1, 0:w])
                        if FP8PV:
                            rr0 = epi.tile([32, ICW], F32, name="rr0", tag="rr0")
                            rr1 = epi.tile([32, ICW], F32, name="rr1", tag="rr1")
                            nc.vector.stream_shuffle(rr0[0:32, 0:w], rt0[0:32, 0:w], [0] * 32)
                            nc.vector.stream_shuffle(rr1[0:32, 0:w], rt1[0:32, 0:w], [0] * 32)
                            nc.vector.tensor_mul(oc0[0:32, ic0:ic0 + w], pv0[0:32, 0:w], rr0[0:32, 0:w])
                            nc.vector.tensor_mul(oc1[0:32, ic0:ic0 + w], pv1[0:32, 0:w], rr1[0:32, 0:w])
                        if DEBUG_STAGE >= 4 and FP8PV and ic0 + w == NQ:
                            g, mt0 = divmod(hg * 4 + 2 * pr, 4)
                            for hl in range(2):
                                nc.sync.dma_start(
                                    od_sb[g][(mt0 + hl) * 32:(mt0 + hl + 1) * 32, :],
                                    outc_sb[hg * 4 + 2 * pr + hl][0:32, :])
                        else:
                            rr = epi.tile([128, ICW], F32, name="rr", tag="rr")
                            rrb = epi.tile([32, ICW], F32, name="rrb", tag="rrb")
                            nc.vector.stream_shuffle(rr[0:32, 0:w], rt0[0:32, 0:w], [0] * 32)
                            nc.vector.stream_shuffle(rrb[0:32, 0:w], rt1[0:32, 0:w], [0] * 32)
                            nc.vector.tensor_copy(rr[64:96, 0:w], rrb[0:32, 0:w])
                            nc.vector.tensor_mul(oc[0:32, ic0:ic0 + w], pv[0:32, 0:w], rr[0:32, 0:w])
                            nc.vector.tensor_mul(oc[64:96, ic0:ic0 + w], pv[64:96, 0:w], rr[64:96, 0:w])
                        if DEBUG_STAGE >= 4 and not FP8PV and ic0 + w == NQ:
                            g = hg
                            nc.sync.dma_start(od_sb[g][pr * 64:pr * 64 + 32, :], oc[0:32, :])
                            nc.sync.dma_start(od_sb[g][pr * 64 + 32:pr * 64 + 64, :], oc[64:96, :])

        if DEBUG_STAGE < 4:
            for g in range(2):
                nc.vector.tensor_copy(y_sb[g][:], outc_sb[g][:])
                nc.sync.dma_start(y_d[g * 128:(g + 1) * 128, :], y_sb[g][:])
            return

        # ---------------- output projection ----------------
        # od gather DMAs were emitted per-pair inside the attention loop.
        # chunk-outer so the wide chunks' y DMAs (gpsimd: casts bf16->f32)
        # overlap the remaining matmuls; narrow chunk drains last.
        with tc.tile_pool(name="fin", bufs=2, space="PSUM") as fin:
            for (o, w) in _chunks(NQ, 512):
                for mt in range(2):
                    msl = slice(mt * 128, (mt + 1) * 128)
                    fps = fin.tile([128, 512], F32, name="fps", tag="fps")
                    nc.tensor.matmul(fps[:, :w], w0_sb[0][:, msl], od_sb[0][:, o:o + w], start=True, stop=False)
                    nc.tensor.matmul(fps[:, :w], w0_sb[1][:, msl], od_sb[1][:, o:o + w], start=False, stop=True)
                    nc.scalar.activation(y_sb[mt][:, o:o + w], fps[:, :w], AF.Identity,
                                         bias=w0p_sb[mt][:, 0:1])
                    nc.sync.dma_start(y_d[msl, o:o + w], y_sb[mt][:, o:o + w])


def build_program():
    nc = bacc.Bacc(
        "TRN2",
        target_bir_lowering=False,
        debug=False,
        enable_asserts=False,
        num_devices=NCORES,
    )
    x_d = nc.dram_tensor("x", [C, N], IN_DT, kind="ExternalInput").ap()
    wq_d = nc.dram_tensor("wq", [C, C], IN_DT, kind="ExternalInput").ap()
    bq_d = nc.dram_tensor("bq", [C, 1], F32, kind="ExternalInput").ap()
    wk_d = nc.dram_tensor("wk", [C, C], IN_DT, kind="ExternalInput").ap()
    bk_d = nc.dram_tensor("bk", [C, 1], F32, kind="ExternalInput").ap()
    wv_d = nc.dram_tensor("wv", [C, NV], IN_DT, kind="ExternalInput").ap()
    wvb_d = nc.dram_tensor("wvb", [128, NV], F32, kind="ExternalInput").ap()
    w0_d = nc.dram_tensor("w0", [C, C], OUT_DT, kind="ExternalInput").ap()
    w0b_d = nc.dram_tensor("w0b", [C, 1], F32, kind="ExternalInput").ap()
    y_d = nc.dram_tensor("y", [C, NQ], F32, kind="ExternalOutput").ap()

    with tile.TileContext(nc) as tc:
        _body(tc, x_d, wq_d, bq_d, wk_d, bk_d, wv_d, wvb_d, w0_d, w0b_d, y_d)
    nc.compile()
    return nc


_CACHE = {}


def _get_program():
    if "nc" not in _CACHE:
        _CACHE["nc"] = build_program()
    return _CACHE["nc"]


def make_in_maps(x, Wqkv, bqkv, W0, b0):
    f = np.float32
    x = np.asarray(x, f)
    Wqkv = np.asarray(Wqkv, f)
    bqkv = np.asarray(bqkv, f)
    W0 = np.asarray(W0, f)
    b0 = np.asarray(b0, f)

    scale = f(D) ** f(-0.5)
    # channel o = d*24 + k*8 + m ; column layout is head-major (m, d) -> m*32+d
    md = (np.arange(HEADS)[:, None] + 24 * np.arange(D)[None, :]).reshape(-1)
    q_rows, k_rows, v_rows = md + 0, md + 8, md + 16

    wq = np.ascontiguousarray((Wqkv[q_rows, :] * scale).T, dtype=f)
    bq = np.ascontiguousarray((bqkv[q_rows] * scale).reshape(-1, 1), dtype=f)
    wk = np.ascontiguousarray(Wqkv[k_rows, :].T, dtype=f)
    bk = np.ascontiguousarray(bqkv[k_rows].reshape(-1, 1), dtype=f)

    wv = np.zeros((C, NV), f)
    wvb = np.zeros((1, NV), f)
    for m in range(HEADS):
        vr = v_rows[m * D:(m + 1) * D]
        wv[0:C, m * 34:m * 34 + 32] = Wqkv[vr, :].T
        wvb[0, m * 34:m * 34 + 32] = bqkv[vr]
        wvb[0, m * 34 + 32] = 1.0
    wvb = np.ascontiguousarray(np.repeat(wvb, 128, axis=0))

    w0 = np.ascontiguousarray(W0.T, dtype=f)  # [c, o], c rows head-major
    w0b = np.ascontiguousarray(b0[:, None], dtype=f)

    np_in = mybir.dt.np(IN_DT)
    np_out = mybir.dt.np(OUT_DT)
    shared = {"wq": wq.astype(np_in), "bq": bq, "wk": wk.astype(np_in),
              "bk": bk, "wv": wv.astype(np_in), "wvb": wvb,
              "w0": w0.astype(np_out), "w0b": w0b}
    maps = []
    for b in range(B):
        xb = x[b].reshape(C, N)
        for half in range(2):
            if half == 0:
                xp = xb
            else:
                xp = np.concatenate([xb[:, NQ:], xb[:, :NQ]], axis=1)
            maps.append({"x": np.ascontiguousarray(xp).astype(np_in), **shared})
    return maps


def assemble_output(ys):
    out = np.empty((B, C, N), np.float32)
    for b in range(B):
        out[b][:, 0:NQ] = ys[2 * b]
        out[b][:, NQ:] = ys[2 * b + 1]
    return out.reshape(B, C, HH, WW)


def run(inputs, trace=False):
    nc = _get_program()
    maps = make_in_maps(**inputs)
    res = bass_utils.run_bass_kernel_spmd(
        nc, maps, core_ids=list(range(NCORES)), trace=trace
    )
    ys = [res.results[c]["y"] for c in range(NCORES)]
    return assemble_output(ys), res.exec_time_ns


def kernel(**inputs):
    out, _ = run(inputs, trace=False)
    return out



# revision 50
# speedup vs baseline: 1.0077x; 1.0077x over previous
"""Multi-head self-attention (1x1-conv QKV -> softmax attention -> 1x1-conv)
on Trainium2, 8 NeuronCores, data-parallel over (batch, query-half).

Problem (hardcoded): x[4,256,48,48], Wqkv[768,256], bqkv[768], W0[256,256],
b0[256]; heads=8, dim_head=32, n=2304 pixels.

Sharding: core = b*2 + half. Each core computes K/V for its whole image
(2304 keys) and attention + output projection for its 1152 queries.
No cross-core communication.

Per-core dataflow (bf16 matmul operands, fp32 PSUM accumulation —
HW-measured: bf16 is the only dtype that streams the PE at full rate;
fp32 is 2-pass and float32r single-pass runs at half clock, both ~2x
slower; fp8 DoubleRow PV measured no faster and rel-err ~1.7e-2):
  - x [256, 2304] bf16 (query half permuted first), weights bf16.
  - k_all [(m,d)=256, j] / q_all [(m,d)=256, i]: K=128-accum matmuls,
    bias via ACT Identity on the PSUM->SBUF copy (Wq,bq pre-scaled by
    d^-0.5 on host).
  - vT [j, 8*(32+2)]: per head 32 v-dims + ones col (den trick) + pad;
    V bias added via DVE tensor_add with a host-broadcast [128,NV] tile.
  - scores^T S_T[j, i] = k_m^T q_m per head: K=32 matmuls, head PAIRS
    row-packed via tile_position (rb, rb+32) — the pair runs
    CONCURRENTLY in different PE row groups; each matmul's output owns
    a full PSUM bank.  Query chunks (512, 512, 128) — narrow chunk last
    so the end-of-kernel pipeline drain is short.
  - P = exp(S_T) on ScalarE (ACT), bf16 out (no max subtraction: |s|<~6).
    (A custom-DVE polynomial exp (KDVE env) exists but measured slower:
    its 2.3us latency stalls the PE pipeline.)
  - out^T+den = [vT | 1]^T @ P: M=34 matmuls col-packed via tile_position
    (0,0)/(0,64) into one PSUM bank — the head pair again overlaps in
    different PE column groups.  Accumulate over the 18 key tiles; psum
    row 32 (head0) / 96 (head1) = softmax denominators.
  - normalize: den rows copied to partition 0, reciprocal_approx_fast,
    stream_shuffle broadcast across the 32-block, DVE multiply into outc
    (all DVE ops base-aligned).  od gather (SBUF->SBUF partition-remap
    DMA) emitted per pair right after its last chunk so it overlaps the
    remaining attention.
  - y = W0 @ od + b0: K=128-accum bf16 matmuls; b0 applied as ACT
    per-partition bias on the PSUM->SBUF copy.
Engine balance at ~234us: PE ~206us busy (out-column-rate bound:
QK+PV stream 332k psum columns ~= 138us floor + weight loads), ACT
~197us (softmax exp, 1 elem/cycle/partition floor = 138us), DVE ~53us.
Both engines sit at their structural floors; y written per chunk on the
sync DMA queue (gpsimd casting DMAs add a ~3.4us DGE drain at teardown
— avoided), prologue weight loads on the scalar DMA queue.
"""

import os as _os

import numpy as np

import concourse.bass as bass
import concourse.mybir as mybir
import concourse.tile as tile
from concourse import bacc
from concourse import bass_utils

F32 = mybir.dt.float32
BF16 = mybir.dt.bfloat16
R32 = mybir.dt.float32r
AF = mybir.ActivationFunctionType

B, C, HH, WW = 4, 256, 48, 48
HEADS, D = 8, 32
N = HH * WW            # 2304 keys per image
NCORES = 8
NQ = N // 2            # 1152 queries per core
JT = N // 128          # 18 key tiles
ICW = 512              # query chunk width; narrow chunk last (tail drain)
CHUNKS = [(0, 512), (512, 512), (1024, 128)]
NV = HEADS * (D + 2)   # 272: vT columns (32 v dims + ones col + zero pad
                       # per head; f32r matmul lhs free dim must be even)

DEBUG_STAGE = int(_os.environ.get("KSTAGE", "4"))
RECIP_MODE = _os.environ.get("KREC", "fast")
_DTMAP = {"f32": F32, "bf16": BF16, "f32r": R32, "f8e4": mybir.dt.float8e4}
QK_DT = _DTMAP[_os.environ.get("KQK", "bf16")]
PV_DT = _DTMAP[_os.environ.get("KPV", "bf16")]
IN_DT = _DTMAP[_os.environ.get("KIN", "bf16")]   # x / proj weights (matmul ins)
OUT_DT = _DTMAP[_os.environ.get("KOUT", "bf16")]  # outc / od / w0 (final matmul)
FP8PV = PV_DT == mybir.dt.float8e4


def _mm(ap):
    return ap


# ---- custom DVE exp: offload part of the softmax exp to the Vector engine.
# exp(s + b) == (alpha * p(s/32))^32 with p a minimax cubic for e^t on
# |t|<=0.25 and alpha = e^(b/32).  Two DVE ops: cubic+alpha (8 ALU stages),
# then five squarings (5 stages).  Max rel err ~1e-3 over |s|<=8.
DVE_EXP = int(_os.environ.get("KDVE", "0"))   # 0=off, N>0: every Nth j on DVE
_EXP_C2, _EXP_C3 = 0.5020893755368885, 0.16666274457238633
_DVE_OPS = {}


def _register_dve_exp():
    if _DVE_OPS:
        return _DVE_OPS
    from concourse.dve_spec import (
        Spec, Src0, C0, C1, C2, C3, One, sq, lower, _spill_c3_to_src1,
        _has_src1,
    )
    from concourse import dve_ops as dops
    from concourse.dve_uop import DveOpSpec

    def mk(name, spec):
        if name in dops._SUB_OPCODE_FOR_NAME:
            for op in dops.OPS:
                if op.name == name:
                    return op
        opcode = dops._CUSTOM_DVE_ROW_BASE + len(dops.OPS)
        shas = {}
        for ver in ("v3", "v4"):
            shas[ver] = DveOpSpec(
                name=name, opcode=opcode, uops=lower(spec, ver=ver),
                rd1_en=_has_src1(spec),
            ).sha(ver)
        op = dops.DveOp(name, spec, subdim=False, uops_sha=shas)
        dops.OPS.append(op)
        dops._SUB_OPCODE_FOR_NAME[name] = opcode
        return op

    t = Src0 * C0
    p = ((t * C1 + C2) * t + One) * t + One

    def ref1(in0, in1, s0, s1, imm2):
        tt = in0.astype(np.float32) * s0
        return (((tt * s1 + imm2) * tt + 1.0) * tt + 1.0) * in1

    op1 = mk("MHSA_EXPP", Spec(body=_spill_c3_to_src1(p * C3), reference=ref1))
    op2 = mk("MHSA_EXPS", Spec(
        body=sq(sq(sq(sq(sq(Src0))))),
        reference=lambda in0, in1, s0, s1, imm2: in0.astype(np.float32) ** 32,
    ))
    _DVE_OPS.update(op1=op1, op2=op2)
    return _DVE_OPS


def _chunks(total, step):
    out = []
    o = 0
    while o < total:
        w = min(step, total - o)
        out.append((o, w))
        o += w
    return out


def _body(tc, x_d, wq_d, bq_d, wk_d, bk_d, wv_d, wvb_d, w0_d, w0b_d, y_d):
    from contextlib import ExitStack

    nc = tc.nc
    with ExitStack() as ctx:
        const = ctx.enter_context(tc.tile_pool(name="const", bufs=1))
        data = ctx.enter_context(tc.tile_pool(name="data", bufs=1))

        # ---------------- load inputs ----------------
        x_sb = [const.tile([128, N], IN_DT, name=f"xa{t}", tag=f"xa{t}") for t in range(2)]

        def load2(name, dram, cols, dt=IN_DT):
            # row-half 0 on the sync queue, row-half 1 on the scalar queue:
            # descriptor generation serializes per queue, so mirroring the
            # halves across two queues halves every load's latency.
            ts_ = [const.tile([128, cols], dt, name=f"{name}{t}", tag=f"{name}{t}") for t in range(2)]
            nc.sync.dma_start(ts_[0][:], dram[0:128, :])
            nc.scalar.dma_start(ts_[1][:], dram[128:256, :])
            return ts_

        # issue order matters: the first K-proj matmul needs wk, x chunk 0
        # and bk — put those first, and split the two x row-halves across
        # the sync and vector DMA queues so descriptor generation runs in
        # parallel (it serializes per issuing queue).
        wk_sb = load2("wk", wk_d, C)
        nc.sync.dma_start(x_sb[0][:, 0:768], x_d[0:128, 0:768])
        nc.scalar.dma_start(x_sb[1][:, 0:768], x_d[128:256, 0:768])
        bk_sb = load2("bk", bk_d, 1, dt=F32)
        for (o, w) in _chunks(N - 768, 768):
            nc.sync.dma_start(x_sb[0][:, 768 + o:768 + o + w], x_d[0:128, 768 + o:768 + o + w])
            nc.scalar.dma_start(x_sb[1][:, 768 + o:768 + o + w], x_d[128:256, 768 + o:768 + o + w])
        wq_sb = load2("wq", wq_d, C)
        bq_sb = load2("bq", bq_d, 1, dt=F32)
        wv_sb = load2("wv", wv_d, NV)
        wvb_sb = const.tile([128, NV], F32, name="wvbias", tag="wvbias")
        nc.sync.dma_start(wvb_sb[:], wvb_d[:, :])
        w0_sb = load2("w0", w0_d, C, dt=OUT_DT)
        w0p_sb = load2("w0p", w0b_d, 1, dt=F32)

        # reciprocal staging tiles: rows 1-31 stay 1.0 forever (only row 0 is
        # rewritten by the per-chunk reciprocal), so memset once here.
        rt0 = const.tile([32, ICW], F32, name="rt0", tag="rt0")
        rt1 = const.tile([32, ICW], F32, name="rt1", tag="rt1")
        nc.vector.memset(rt0[:], 1.0)
        nc.vector.memset(rt1[:], 1.0)
        expb_sb = const.tile([128, 1], F32, name="expb", tag="expb")
        nc.vector.memset(expb_sb[:], -2.0)
        if DVE_EXP:
            _register_dve_exp()
            alpha_sb = const.tile([128, 1], F32, name="alpha", tag="alpha")
            nc.vector.memset(alpha_sb[:], float(np.exp(-2.0 / 32.0)) if FP8PV else 1.0)

        # persistent activations
        k_sb = [data.tile([128, N], QK_DT, name=f"k{g}", tag=f"k{g}") for g in range(2)]
        q_sb = [data.tile([128, NQ], QK_DT, name=f"q{g}", tag=f"q{g}") for g in range(2)]
        # fp8 DoubleRow path: vt for a j-PAIR lives in one [128, 2*NV] tile
        # (middle dim = j parity) so one matmul contracts 256 keys.
        # exp shift (see expb_sb): keeps exp(s) inside fp8e4 range
        if FP8PV:
            vt_sb = [data.tile([128, 2 * NV], PV_DT, name=f"vt{jp}", tag=f"vt{jp}")
                     for jp in range(JT // 2)]
        else:
            vt_sb = [data.tile([128, NV], PV_DT, name=f"vt{j}", tag=f"vt{j}") for j in range(JT)]
        # output tiles: fp8 path keeps one [32, NQ] tile per head (PV psum at
        # partition base 0); bf16 path keeps the packed pv layout (head pair
        # at partitions 0-31 / 64-95 of a [128, NQ] tile).
        if FP8PV:
            outc_sb = [data.tile([32, NQ], OUT_DT, name=f"oc{t}", tag=f"oc{t}")
                       for t in range(HEADS)]
        else:
            outc_sb = [data.tile([128, NQ], OUT_DT, name=f"oc{t}", tag=f"oc{t}")
                       for t in range(4)]
        y_sb = [data.tile([128, NQ], F32, name=f"y{g}", tag=f"y{g}") for g in range(2)]

        # ---------------- projections ----------------
        with tc.tile_pool(name="prj", bufs=2, space="PSUM") as prj:
            for hg in range(2):
                hsl = slice(hg * 128, (hg + 1) * 128)
                for (o, w) in _chunks(N, 512):
                    kps = prj.tile([128, 512], F32, name="kps", tag="kps")
                    nc.tensor.matmul(kps[:, :w], wk_sb[0][:, hsl], x_sb[0][:, o:o + w], start=True, stop=False)
                    nc.tensor.matmul(kps[:, :w], wk_sb[1][:, hsl], x_sb[1][:, o:o + w], start=False, stop=True)
                    nc.scalar.activation(k_sb[hg][:, o:o + w], kps[:, :w], AF.Identity, bias=bk_sb[hg][:, 0:1])
                for (o, w) in _chunks(NQ, 512):
                    qps = prj.tile([128, 512], F32, name="qps", tag="qps")
                    nc.tensor.matmul(qps[:, :w], wq_sb[0][:, hsl], x_sb[0][:, o:o + w], start=True, stop=False)
                    nc.tensor.matmul(qps[:, :w], wq_sb[1][:, hsl], x_sb[1][:, o:o + w], start=False, stop=True)
                    nc.scalar.activation(q_sb[hg][:, o:o + w], qps[:, :w], AF.Identity, bias=bq_sb[hg][:, 0:1])
            for j in range(JT):
                jsl = slice(j * 128, (j + 1) * 128)
                vps = prj.tile([128, NV], F32, name="vps", tag="vps")
                nc.tensor.matmul(vps[:], x_sb[0][:, jsl], wv_sb[0][:], start=True, stop=False)
                nc.tensor.matmul(vps[:], x_sb[1][:, jsl], wv_sb[1][:], start=False, stop=True)
                if FP8PV:
                    jo = j % 2
                    nc.vector.tensor_add(
                        vt_sb[j // 2][:, jo * NV:(jo + 1) * NV], vps[:], wvb_sb[:])
                else:
                    nc.vector.tensor_add(vt_sb[j][:], vps[:], wvb_sb[:])

        if DEBUG_STAGE < 2:
            for g in range(2):
                nc.vector.tensor_copy(y_sb[g][:], q_sb[g][:])
                nc.sync.dma_start(y_d[g * 128:(g + 1) * 128, :], y_sb[g][:])
            return

        od_sb = [data.tile([128, NQ], OUT_DT, name=f"od{g}", tag=f"od{g}") for g in range(2)]

        # ---------------- attention main loop ----------------
        # PSUM budget: st (3 or 2) bufs x 2 banks + pv banks = 8.
        STB = 2 if FP8PV else 3
        with tc.tile_pool(name="stp", bufs=STB, space="PSUM") as stp, \
             tc.tile_pool(name="pvp", bufs=2, space="PSUM") as pvp, \
             tc.tile_pool(name="pv1p", bufs=2, space="PSUM") as pv1p, \
             tc.tile_pool(name="ptp", bufs=4) as ptp, \
             tc.tile_pool(name="etp", bufs=2) as etp, \
             tc.tile_pool(name="epi", bufs=2) as epi:
            for hg in range(2):
                for pr in range(2):
                    rb = pr * 64       # partition base of this head pair
                    for (ic0, w) in CHUNKS:
                        # head pair packed into one PSUM bank via column
                        # tile_position: head0 at partitions 0:34, head1 at
                        # 64:98 — the two matmuls run concurrently in
                        # different PE column groups.
                        if FP8PV:
                            pv0 = pvp.tile([34, ICW], F32, name="pv0", tag="pv0")
                            pv1 = pv1p.tile([34, ICW], F32, name="pv1", tag="pv1")
                            pvs = (pv0, pv1)
                        else:
                            pv = pvp.tile([128, ICW], F32, name="pv", tag="pv")
                        pts = {}

                        if FP8PV:
                            # one DoubleRow matmul per j-pair: lhsT/rhs get a
                            # middle dim of 2 (j parity), contracting 256 keys.
                            def emit_pv(jp, w=w, pvs=pvs, pts=pts, hg=hg, pr=pr):
                                pt = pts.pop(jp)
                                p3 = pt[:].rearrange("p (o h q) -> p o h q", o=2, h=2)
                                v3 = vt_sb[jp][:].rearrange("p (o v) -> p o v", o=2)
                                for hl in range(2):
                                    gh = hg * 4 + 2 * pr + hl
                                    nc.tensor.matmul(
                                        pvs[hl][0:34, 0:w],
                                        v3[:, :, gh * 34:gh * 34 + 34],
                                        p3[:, :, hl, 0:w],
                                        start=(jp == 0), stop=(jp == JT // 2 - 1),
                                        perf_mode=mybir.MatmulPerfMode.DoubleRow,
                                    )
                        else:
                            def emit_pv(j, w=w, pv=pv, pts=pts, hg=hg, pr=pr):
                                pt = pts.pop(j)
                                for hl, base in enumerate((0, 64)):
                                    gh = hg * 4 + 2 * pr + hl
                                    nc.tensor.matmul(
                                        pv[base:base + 34, 0:w],
                                        _mm(vt_sb[j][:, gh * 34:gh * 34 + 34]),
                                        _mm(pt[:, hl * ICW:hl * ICW + w]),
                                        start=(j == 0), stop=(j == JT - 1),
                                        tile_position=(0, base),
                                    )

                        for j in range(JT):
                            st = stp.tile([128, 1024], F32, name="st", tag="st")
                            for hl in range(2):
                                nc.tensor.matmul(
                                    st[:, hl * 512:hl * 512 + w],
                                    _mm(k_sb[hg][rb + hl * 32:rb + (hl + 1) * 32, j * 128:(j + 1) * 128]),
                                    _mm(q_sb[hg][rb + hl * 32:rb + (hl + 1) * 32, ic0:ic0 + w]),
                                    start=True, stop=True,
                                    tile_position=(rb + hl * 32, 0),
                                )
                            use_dve = DVE_EXP > 0 and (j % DVE_EXP == DVE_EXP - 1)
                            src_v = st[:].rearrange("p (s q) -> p s q", s=2)[:, :, 0:w]
                            if FP8PV:
                                jo = j % 2
                                if jo == 0:
                                    pt = ptp.tile([128, 4 * ICW], PV_DT, name="pt", tag="pt")
                                    pts[j // 2] = pt
                                else:
                                    pt = pts[j // 2]
                                dst_v = (pt[:, jo * 2 * ICW:(jo + 1) * 2 * ICW]
                                         .rearrange("p (s q) -> p s q", s=2)[:, :, 0:w])
                            else:
                                pt = ptp.tile([128, 2 * ICW], PV_DT, name="pt", tag="pt")
                                dst_v = pt[:].rearrange("p (s q) -> p s q", s=2)[:, :, 0:w]
                            if use_dve:
                                et = etp.tile([128, 1024], F32, name="et", tag="et")
                                et_v = et[:].rearrange("p (s q) -> p s q", s=2)[:, :, 0:w]
                                nc.vector._custom_dve(
                                    _DVE_OPS["op1"], out=et_v, in0=src_v,
                                    in1=alpha_sb[:, 0:1], s0=1.0 / 32.0,
                                    s1=_EXP_C3, imm2=_EXP_C2)
                                nc.vector._custom_dve(
                                    _DVE_OPS["op2"], out=dst_v, in0=et_v)
                            elif FP8PV:
                                nc.scalar.activation(dst_v, src_v, AF.Exp, bias=expb_sb[:])
                            else:
                                nc.scalar.activation(dst_v, src_v, AF.Exp)
                            if FP8PV:
                                if jo == 1 and j // 2 >= 1:
                                    emit_pv(j // 2 - 1)
                            else:
                                pts[j] = pt
                                if j >= 1:
                                    emit_pv(j - 1)
                        emit_pv(JT - 1 if not FP8PV else JT // 2 - 1)

                        # epilogue: reciprocal of the den row at partition 0,
                        # stream_shuffle broadcast across the 32-block, DVE
                        # multiply into outc (all ops base-aligned).
                        dt0 = epi.tile([1, ICW], F32, name="dt0", tag="dt0")
                        dt1 = epi.tile([1, ICW], F32, name="dt1", tag="dt1")
                        if FP8PV:
                            oc0 = outc_sb[hg * 4 + 2 * pr]
                            oc1 = outc_sb[hg * 4 + 2 * pr + 1]
                            if DEBUG_STAGE < 3:
                                nc.vector.tensor_copy(oc0[0:32, ic0:ic0 + w], pv0[0:32, 0:w])
                                nc.vector.tensor_copy(oc1[0:32, ic0:ic0 + w], pv1[0:32, 0:w])
                                continue
                            nc.vector.tensor_copy(dt0[0:1, 0:w], pv0[32:33, 0:w])
                            nc.vector.tensor_copy(dt1[0:1, 0:w], pv1[32:33, 0:w])
                        else:
                            oc = outc_sb[hg * 2 + pr]
                            if DEBUG_STAGE < 3:
                                nc.vector.tensor_copy(oc[0:33, ic0:ic0 + w], pv[0:33, 0:w])
                                nc.vector.tensor_copy(oc[64:97, ic0:ic0 + w], pv[64:97, 0:w])
                                continue
                            nc.vector.tensor_copy(dt0[0:1, 0:w], pv[32:33, 0:w])
                            nc.vector.tensor_copy(dt1[0:1, 0:w], pv[96:97, 0:w])
                        if RECIP_MODE == "fast":
                            nc.vector.reciprocal_approx_fast(rt0[0:1, 0:w], dt0[0:1, 0:w])
                            nc.vector.reciprocal_approx_fast(rt1[0:1, 0:w], dt1[0:1, 0:w])
                        else:
                            nc.vector.reciprocal(rt0[0:1, 0:w], dt0[0:1, 0:w])
                            nc.vector.reciprocal(rt1[0:1, 0:w], dt1[0:1, 0:w])
                        if FP8PV:
                            rr0 = epi.tile([32, ICW], F32, name="rr0", tag="rr0")
                            rr1 = epi.tile([32, ICW], F32, name="rr1", tag="rr1")
                            nc.vector.stream_shuffle(rr0[0:32, 0:w], rt0[0:32, 0:w], [0] * 32)
                            nc.vector.stream_shuffle(rr1[0:32, 0:w], rt1[0:32, 0:w], [0] * 32)
                            nc.vector.tensor_mul(oc0[0:32, ic0:ic0 + w], pv0[0:32, 0:w], rr0[0:32, 0:w])
                            nc.vector.tensor_mul(oc1[0:32, ic0:ic0 + w], pv1[0:32, 0:w], rr1[0:32, 0:w])
                        if DEBUG_STAGE >= 4 and FP8PV and ic0 + w == NQ:
                            g, mt0 = divmod(hg * 4 + 2 * pr, 4)
                            for hl in range(2):
                                nc.sync.dma_start(
                                    od_sb[g][(mt0 + hl) * 32:(mt0 + hl + 1) * 32, :],
                                    outc_sb[hg * 4 + 2 * pr + hl][0:32, :])
                        else:
                            rr = epi.tile([128, ICW], F32, name="rr", tag="rr")
                            rrb = epi.tile([32, ICW], F32, name="rrb", tag="rrb")
                            nc.vector.stream_shuffle(rr[0:32, 0:w], rt0[0:32, 0:w], [0] * 32)
                            nc.vector.stream_shuffle(rrb[0:32, 0:w], rt1[0:32, 0:w], [0] * 32)
                            nc.vector.tensor_copy(rr[64:96, 0:w], rrb[0:32, 0:w])
                            nc.vector.tensor_mul(oc[0:32, ic0:ic0 + w], pv[0:32, 0:w], rr[0:32, 0:w])
                            nc.vector.tensor_mul(oc[64:96, ic0:ic0 + w], pv[64:96, 0:w], rr[64:96, 0:w])
                        if DEBUG_STAGE >= 4 and not FP8PV and ic0 + w == NQ:
                            g = hg
                            nc.sync.dma_start(od_sb[g][pr * 64:pr * 64 + 32, :], oc[0:32, :])
                            nc.sync.dma_start(od_sb[g][pr * 64 + 32:pr * 64 + 64, :], oc[64:96, :])

        if DEBUG_STAGE < 4:
            for g in range(2):
                nc.vector.tensor_copy(y_sb[g][:], outc_sb[g][:])
                nc.sync.dma_start(y_d[g * 128:(g + 1) * 128, :], y_sb[g][:])
            return

        # ---------------- output projection ----------------
        # od gather DMAs were emitted per-pair inside the attention loop.
        # chunk-outer so the wide chunks' y DMAs (gpsimd: casts bf16->f32)
        # overlap the remaining matmuls; narrow chunk drains last.
        with tc.tile_pool(name="fin", bufs=2, space="PSUM") as fin:
            for (o, w) in _chunks(NQ, 512):
                for mt in range(2):
                    msl = slice(mt * 128, (mt + 1) * 128)
                    fps = fin.tile([128, 512], F32, name="fps", tag="fps")
                    nc.tensor.matmul(fps[:, :w], w0_sb[0][:, msl], od_sb[0][:, o:o + w], start=True, stop=False)
                    nc.tensor.matmul(fps[:, :w], w0_sb[1][:, msl], od_sb[1][:, o:o + w], start=False, stop=True)
                    nc.scalar.activation(y_sb[mt][:, o:o + w], fps[:, :w], AF.Identity,
                                         bias=w0p_sb[mt][:, 0:1])
                    nc.sync.dma_start(y_d[msl, o:o + w], y_sb[mt][:, o:o + w])


def build_program():
    nc = bacc.Bacc(
        "TRN2",
        target_bir_lowering=False,
        debug=False,
        enable_asserts=False,
        num_devices=NCORES,
    )
    x_d = nc.dram_tensor("x", [C, N], IN_DT, kind="ExternalInput").ap()
    wq_d = nc.dram_tensor("wq", [C, C], IN_DT, kind="ExternalInput").ap()
    bq_d = nc.dram_tensor("bq", [C, 1], F32, kind="ExternalInput").ap()
    wk_d = nc.dram_tensor("wk", [C, C], IN_DT, kind="ExternalInput").ap()
    bk_d = nc.dram_tensor("bk", [C, 1], F32, kind="ExternalInput").ap()
    wv_d = nc.dram_tensor("wv", [C, NV], IN_DT, kind="ExternalInput").ap()
    wvb_d = nc.dram_tensor("wvb", [128, NV], F32, kind="ExternalInput").ap()
    w0_d = nc.dram_tensor("w0", [C, C], OUT_DT, kind="ExternalInput").ap()
    w0b_d = nc.dram_tensor("w0b", [C, 1], F32, kind="ExternalInput").ap()
    y_d = nc.dram_tensor("y", [C, NQ], F32, kind="ExternalOutput").ap()

    with tile.TileContext(nc) as tc:
        _body(tc, x_d, wq_d, bq_d, wk_d, bk_d, wv_d, wvb_d, w0_d, w0b_d, y_d)
    nc.compile()
    return nc


_CACHE = {}


def _get_program():
    if "nc" not in _CACHE:
        _CACHE["nc"] = build_program()
    return _CACHE["nc"]


def make_in_maps(x, Wqkv, bqkv, W0, b0):
    f = np.float32
    x = np.asarray(x, f)
    Wqkv = np.asarray(Wqkv, f)
    bqkv = np.asarray(bqkv, f)
    W0 = np.asarray(W0, f)
    b0 = np.asarray(b0, f)

    scale = f(D) ** f(-0.5)
    # channel o = d*24 + k*8 + m ; column layout is head-major (m, d) -> m*32+d
    md = (np.arange(HEADS)[:, None] + 24 * np.arange(D)[None, :]).reshape(-1)
    q_rows, k_rows, v_rows = md + 0, md + 8, md + 16

    wq = np.ascontiguousarray((Wqkv[q_rows, :] * scale).T, dtype=f)
    bq = np.ascontiguousarray((bqkv[q_rows] * scale).reshape(-1, 1), dtype=f)
    wk = np.ascontiguousarray(Wqkv[k_rows, :].T, dtype=f)
    bk = np.ascontiguousarray(bqkv[k_rows].reshape(-1, 1), dtype=f)

    wv = np.zeros((C, NV), f)
    wvb = np.zeros((1, NV), f)
    for m in range(HEADS):
        vr = v_rows[m * D:(m + 1) * D]
        wv[0:C, m * 34:m * 34 + 32] = Wqkv[vr, :].T
        wvb[0, m * 34:m * 34 + 32] = bqkv[vr]
        wvb[0, m * 34 + 32] = 1.0
    wvb = np.ascontiguousarray(np.repeat(wvb, 128, axis=0))

    w0 = np.ascontiguousarray(W0.T, dtype=f)  # [c, o], c rows head-major
    w0b = np.ascontiguousarray(b0[:, None], dtype=f)

    np_in = mybir.dt.np(IN_DT)
    np_out = mybir.dt.np(OUT_DT)
    shared = {"wq": wq.astype(np_in), "bq": bq, "wk": wk.astype(np_in),
              "bk": bk, "wv": wv.astype(np_in), "wvb": wvb,
              "w0": w0.astype(np_out), "w0b": w0b}
    maps = []
    for b in range(B):
        xb = x[b].reshape(C, N)
        for half in range(2):
            if half == 0:
                xp = xb
            else:
                xp = np.concatenate([xb[:, NQ:], xb[:, :NQ]], axis=1)
            maps.append({"x": np.ascontiguousarray(xp).astype(np_in), **shared})
    return maps


def assemble_output(ys):
    out = np.empty((B, C, N), np.float32)
    for b in range(B):
        out[b][:, 0:NQ] = ys[2 * b]
        out[b][:, NQ:] = ys[2 * b + 1]
    return out.reshape(B, C, HH, WW)


def run(inputs, trace=False):
    nc = _get_program()
    maps = make_in_maps(**inputs)
    res = bass_utils.run_bass_kernel_spmd(
        nc, maps, core_ids=list(range(NCORES)), trace=trace
    )
    ys = [res.results[c]["y"] for c in range(NCORES)]
    return assemble_output(ys), res.exec_time_ns


def kernel(**inputs):
    out, _ = run(inputs, trace=False)
    return out

